# revision 3
# baseline (speedup 1.0000x reference)
# Trainium2 Bass kernel for nn_DiffNet — transposed (feature-major) layout.
#
# Math (same folding as before): with coef = (conv2_w @ conv1_w)[0] = (c0,c1,c2),
# bc = conv2_w @ conv1_b + conv2_b, scale = RATE/batch_num, each layer reduces to
#   P  = vi' @ W.T + bhat          (bias folded exactly via m-offset machinery)
#   out' = alpha(b) * relu(P) + C1*P + delta(b)  (+ m4(o) on the last layer)
# where alpha = C2*s + ka, delta = C0*q + Cb*s + cross + kd, s/q/cross are
# per-batch-row reductions of vi'.
#
# Layout: everything lives feature-major.  P.T [nout, B] is produced directly
# in PSUM by making the WEIGHTS the stationary operand: for each 128-row
# o-chunk c and 128-col k-chunk k,
#   matmul(P.T[c][:, 0:8], lhsT = W.T[k*128:(k+1)*128, c*128:(c+1)*128], rhs = vi.T[k])
# lhsT is a full 128-column fp16 weight tile -> compiler enables FWL (fast
# weight load), so the LDWEIGHTS stream runs at ~2x.  Activations are
# [128, 8] fp16 chunks packed side by side ([128, 32] per layer), so every
# epilogue op runs on 128 partitions (60ns) instead of 8 (700ns), and there
# are no inter-layer transposes at all: the epilogue output IS the next
# layer's rhs.
#
# Per-batch stats come from the PE too: a [128, 3] stationary built on the
# host as [C2*1 | (Cb*1 + 2*C0*m)*2^10 | C0*1] contracted against vi.T and
# square(vi.T) gives rows [C2*s ; 2^10*(Cb*s+cross) ; C0*q] in PSUM.  alpha
# and delta rows [1, 8] are then broadcast down 128 partitions with a
# 1-partition ones-row matmul.
#
# Sharding: data-parallel over batch (64 rows -> 8 per core), weights
# replicated, zero collectives.  Host transposes the [128, 16] result back.

import numpy as np

RATE = 0.01
B, IN, H1, H2, OUT = 64, 1024, 512, 512, 256
NCORES = 8
BL = B // NCORES  # 8 batch rows per core
P128 = 128

NK = [IN // P128, H1 // P128, H2 // P128]   # k-chunks per layer: 8, 4, 4
NCH = [H1 // P128, H2 // P128, OUT // P128]  # o-chunks per layer: 4, 4, 2

# wall (fp16 [128, 7232]): xT pack | L1 W chunks (c-major, k-minor) | L2 | L3
XT_OFF = 0
XT_LEN = NK[0] * BL  # 64
WOFF = [XT_LEN, XT_LEN + 4096, XT_LEN + 6144]
W_LEN = XT_LEN + 7168  # 7232
# DMA slices (tile boundaries must align with matmul chunk reads)
WSLICES = [(0, 2112), (2112, 4160), (4160, 6208), (6208, 7232)]

# c32 (fp32 [128, 32]): bh chunks | cb=C1*bh(+m4) chunks | C1 | zero | ka | kd
BH_OFF = [0, 4, 8]
CB_OFF = [10, 14, 18]
C32_C1 = 20
C32_KA = 22  # cols 22,23,24 (value replicated down the column; [1,1] slices used)
C32_KD = 25  # cols 25,26,27
C32_LEN = 32

# c16 (fp16 [128, 176]): stats lhsT (3 cols per k-chunk) | ones row (row 0)
ST_OFF = [0, 3 * NK[0], 3 * NK[0] + 3 * NK[1]]  # 0, 24, 36
ONES_OFF = 48
C16_LEN = ONES_OFF + P128  # 176

SC_STAT = 1024.0  # prescale on the delta stats column (fp16 underflow guard)

N_WARMUP = 48  # junk matmuls to release the PE HAM clock gate (~2.5us cold)

_NC_CACHE = {}
DEBUG_TAPS = False


def _build_nc():
    import concourse.bacc as bacc
    import concourse.mybir as mybir
    import concourse.tile as tile

    fp32 = mybir.dt.float32
    fp16 = mybir.dt.float16
    AF = mybir.ActivationFunctionType
    ALU = mybir.AluOpType

    nc = bacc.Bacc("TRN2", target_bir_lowering=False, debug=False)

    w_t = nc.dram_tensor("wall", [P128, W_LEN], fp16, kind="ExternalInput")
    c32_t = nc.dram_tensor("c32", [P128, C32_LEN], fp32, kind="ExternalInput")
    c16_t = nc.dram_tensor("c16", [P128, C16_LEN], fp16, kind="ExternalInput")
    out_t = nc.dram_tensor("outT", [P128, 2 * BL], fp32, kind="ExternalOutput")

    with tile.TileContext(nc) as tc:
        with (
            tc.tile_pool(name="wp", bufs=1) as wp,
            tc.tile_pool(name="ap", bufs=1) as ap_,
            tc.tile_pool(name="pp", bufs=2, space="PSUM") as pp,
            tc.tile_pool(name="sp", bufs=2, space="PSUM") as sp,
            tc.tile_pool(name="bp", bufs=2, space="PSUM") as bp,
            tc.tile_pool(name="wmp", bufs=1, space="PSUM") as wmp,
        ):
            # --- DMAs: wall slices on the sync queue, consts on scalar ---
            wseg = []
            for i, (lo, hi) in enumerate(WSLICES):
                t = wp.tile([P128, hi - lo], fp16, tag=f"w{i}")
                nc.sync.dma_start(t[:], w_t[:, lo:hi])
                wseg.append((t, lo))
            c32 = ap_.tile([P128, C32_LEN], fp32, tag="c32")
            nc.scalar.dma_start(c32[:], c32_t[:])
            c16 = ap_.tile([P128, C16_LEN], fp16, tag="c16")
            nc.scalar.dma_start(c16[:], c16_t[:])

            def wall(lo, n):
                for t, off in wseg:
                    if off <= lo and lo + n <= off + t.shape[1]:
                        return t[:, lo - off : lo - off + n]
                raise AssertionError("bad wall slice")

            ones_row = c16[0:1, ONES_OFF : ONES_OFF + P128]

            # --- PE warm-up: junk matmuls release the HAM clock gate while
            # the weight DMA streams in ---
            junk_a = wp.tile([BL, BL], fp16, tag="junk_a")
            junk_w = wp.tile([BL, 64], fp16, tag="junk_w")
            nc.gpsimd.memset(junk_a[:], 0.0)
            nc.gpsimd.memset(junk_w[:], 0.0)
            warm_p = wmp.tile([BL, 64], fp32, tag="warm")
            for _ in range(N_WARMUP):
                nc.tensor.matmul(warm_p[:], junk_a[:], junk_w[:], start=True, stop=True)

            def layer(l, viT, sq):
                """viT: [128, NK[l]*8] fp16 chunk pack; sq: same shape, vi^2.
                Returns (out16 fp16 [128, NCH[l]*8], out32 or None)."""
                nk, nch = NK[l], NCH[l]
                ncol = nch * BL

                # per-batch stats on PE, all landing on partition 0 (engines
                # cannot read at a partition offset): three 1-col stationaries
                # give rows [C2*s | SC*(Cb*s+cross) | C0*q] in [1, 24] PSUM.
                st = sp.tile([1, 3 * BL], fp32, tag="st")
                for j, src in ((0, viT), (1, viT), (2, sq)):
                    for k in range(nk):
                        o = ST_OFF[l] + 3 * k + j
                        nc.tensor.matmul(
                            st[:, j * BL : (j + 1) * BL],
                            c16[:, o : o + 1],
                            src[:, k * BL : (k + 1) * BL],
                            start=(k == 0), stop=(k == nk - 1),
                        )

                # alpha/delta rows [1, 8] (DVE, tiny)
                al = ap_.tile([1, BL], fp16, tag=f"al{l}")
                nc.vector.tensor_scalar(
                    al[:], st[0:1, 0:BL], c32[0:1, C32_KA + l : C32_KA + l + 1],
                    None, ALU.add,
                )
                dtmp = ap_.tile([1, BL], fp32, tag=f"dt{l}")
                nc.vector.tensor_scalar(
                    dtmp[:], st[0:1, BL : 2 * BL], 1.0 / SC_STAT,
                    c32[0:1, C32_KD + l : C32_KD + l + 1], ALU.mult, ALU.add,
                )
                de = ap_.tile([1, BL], fp16, tag=f"de{l}")
                nc.vector.tensor_tensor(
                    de[:], dtmp[:], st[0:1, 2 * BL : 3 * BL], ALU.add
                )

                # P.T = W @ vi' accumulated over k-chunks, per o-chunk
                Pt = pp.tile([P128, ncol], fp32, tag="P")
                for c in range(nch):
                    for k in range(nk):
                        nc.tensor.matmul(
                            Pt[:, c * BL : (c + 1) * BL],
                            wall(WOFF[l] + (c * nk + k) * P128, P128),
                            viT[:, k * BL : (k + 1) * BL],
                            start=(k == 0), stop=(k == nk - 1),
                        )

                # broadcast alpha/delta down the partitions: [128, ncol] each
                bc = bp.tile([P128, 2 * ncol], fp32, tag="bc")
                for c in range(nch):
                    nc.tensor.matmul(
                        bc[:, c * BL : (c + 1) * BL], ones_row, al[:],
                        start=True, stop=True,
                    )
                for c in range(nch):
                    nc.tensor.matmul(
                        bc[:, ncol + c * BL : ncol + (c + 1) * BL], ones_row, de[:],
                        start=True, stop=True,
                    )

                # epilogue, all [128, ncol]-shaped:
                # R = relu(P + bh); t2 = C1*P + (C1*bh + m4); t3 = t2 + deltaB;
                # out = R * alphaB + t3
                R = ap_.tile([P128, ncol], fp32, tag=f"R{l}")
                for c in range(nch):
                    nc.scalar.activation(
                        out=R[:, c * BL : (c + 1) * BL],
                        in_=Pt[:, c * BL : (c + 1) * BL],
                        func=AF.Relu,
                        bias=c32[:, BH_OFF[l] + c : BH_OFF[l] + c + 1],
                    )
                t2 = ap_.tile([P128, ncol], fp32, tag=f"t2{l}")
                for c in range(nch):
                    nc.vector.tensor_scalar(
                        t2[:, c * BL : (c + 1) * BL],
                        Pt[:, c * BL : (c + 1) * BL],
                        c32[:, C32_C1 : C32_C1 + 1],
                        c32[:, CB_OFF[l] + c : CB_OFF[l] + c + 1],
                        ALU.mult, ALU.add,
                    )
                t3 = ap_.tile([P128, ncol], fp32, tag=f"t3{l}")
                nc.vector.tensor_tensor(t3[:], t2[:], bc[:, ncol : 2 * ncol], ALU.add)
                t4 = ap_.tile([P128, ncol], fp32, tag=f"t4{l}")
                nc.vector.tensor_tensor(t4[:], R[:], bc[:, 0:ncol], ALU.mult)

                if l == 2:
                    o32 = ap_.tile([P128, ncol], fp32, tag="o32")
                    nc.vector.tensor_tensor(o32[:], t3[:], t4[:], ALU.add)
                    return None, o32
                o16 = ap_.tile([P128, ncol], fp16, tag=f"o16{l}")
                nc.vector.tensor_tensor(o16[:], t3[:], t4[:], ALU.add)
                return o16, None

            # layer 1 inputs: xT pack + its square
            xT = wall(XT_OFF, XT_LEN)
            sq1 = ap_.tile([P128, XT_LEN], fp16, tag="sq1")
            nc.scalar.activation(out=sq1[:], in_=xT, func=AF.Square)

            o1, _ = layer(0, xT, sq1)
            sq2 = ap_.tile([P128, NCH[0] * BL], fp16, tag="sq2")
            nc.scalar.activation(out=sq2[:], in_=o1[:], func=AF.Square)
            o2, _ = layer(1, o1, sq2)
            sq3 = ap_.tile([P128, NCH[1] * BL], fp16, tag="sq3")
            nc.scalar.activation(out=sq3[:], in_=o2[:], func=AF.Square)
            _, o3 = layer(2, o2, sq3)

            nc.sync.dma_start(out_t[:], o3[:])

            if DEBUG_TAPS:
                for name, ap in (("dbg_o1", o1[:]), ("dbg_o2", o2[:])):
                    t = nc.dram_tensor(
                        name, list(ap.shape), ap.dtype, kind="ExternalOutput"
                    )
                    nc.sync.dma_start(t[:], ap)

    nc.compile()
    return nc


def get_nc():
    if "nc" not in _NC_CACHE:
        _NC_CACHE["nc"] = _build_nc()
    return _NC_CACHE["nc"]


def _wchunks(Wt, nk, nch):
    """[in, out] -> [128, nch*nk*128]: chunk (k, c) at col (c*nk+k)*128."""
    return np.ascontiguousarray(
        Wt.reshape(nk, P128, nch, P128)
        .transpose(1, 2, 0, 3)
        .reshape(P128, nch * nk * P128),
        dtype=np.float16,
    )


def host_prep(x, fc1_w, fc1_b, fc2_w, fc2_b, fc3_w, fc3_b,
              conv1_w, conv1_b, conv2_w, conv2_b, batch_num):
    f32, f16, f64 = np.float32, np.float16, np.float64
    x = np.asarray(x, f32)
    ws = [np.asarray(fc1_w, f32), np.asarray(fc2_w, f32), np.asarray(fc3_w, f32)]
    bs = [np.asarray(fc1_b, f32), np.asarray(fc2_b, f32), np.asarray(fc3_b, f32)]

    bn = float(np.asarray(batch_num).item())
    scale = RATE / bn
    coef = (np.asarray(conv2_w, f64) @ np.asarray(conv1_w, f64))[0]
    bcv = float(
        (np.asarray(conv2_w, f64) @ np.asarray(conv1_b, f64))[0]
        + np.asarray(conv2_b, f64)[0]
    )
    C0, C1, C2 = (scale * coef).astype(f64)
    Cb = scale * bcv

    m2 = (-C1 * bs[0].astype(f64)).astype(f32)
    m3 = (-C1 * bs[1].astype(f64)).astype(f32)
    m4 = (-C1 * bs[2].astype(f64)).astype(f32)
    bh = [
        bs[0],
        (bs[1] + m2 @ ws[1].T).astype(f32),
        (bs[2] + m3 @ ws[2].T).astype(f32),
    ]
    ka = [1.0, 1.0 + C2 * float(m2.sum()), 1.0 + C2 * float(m3.sum())]
    kd = [
        0.0,
        C0 * float(m2 @ m2) + Cb * float(m2.sum()),
        C0 * float(m3 @ m3) + Cb * float(m3.sum()),
    ]
    mvec = [np.zeros(IN, f32), m2, m3]

    c32 = np.zeros((P128, C32_LEN), f32)
    for l in range(3):
        nch = NCH[l]
        bhc = bh[l].reshape(nch, P128).T  # [128, nch]
        c32[:, BH_OFF[l] : BH_OFF[l] + nch] = bhc
        cb = (C1 * bh[l].astype(f64)).astype(f32)
        if l == 2:
            cb = (cb.astype(f64) + m4.astype(f64)).astype(f32)
        c32[:, CB_OFF[l] : CB_OFF[l] + nch] = cb.reshape(nch, P128).T
    c32[:, C32_C1] = C1
    c32[:, C32_KA : C32_KA + 3] = ka
    c32[:, C32_KD : C32_KD + 3] = kd

    c16 = np.zeros((P128, C16_LEN), f16)
    for l in range(3):
        for k in range(NK[l]):
            o = ST_OFF[l] + 3 * k
            c16[:, o] = f16(C2)
            c16[:, o + 1] = (
                SC_STAT * (Cb + 2.0 * C0 * mvec[l][k * P128 : (k + 1) * P128].astype(f64))
            ).astype(f16)
            c16[:, o + 2] = f16(C0)
    c16[0, ONES_OFF : ONES_OFF + P128] = 1.0

    wall_base = np.empty((P128, W_LEN), f16)
    for l in range(3):
        wall_base[:, WOFF[l] : WOFF[l] + NCH[l] * NK[l] * P128] = _wchunks(
            ws[l].T, NK[l], NCH[l]
        )

    in_maps = []
    for k in range(NCORES):
        xk = x[k * BL : (k + 1) * BL]  # [8, 1024]
        wall = wall_base.copy()
        # xT chunks: col kk*8 + b = x[b, kk*128 + p]
        wall[:, XT_OFF : XT_OFF + XT_LEN] = (
            xk.T.reshape(NK[0], P128, BL).transpose(1, 0, 2).reshape(P128, XT_LEN)
        ).astype(f16)
        in_maps.append({"wall": wall, "c32": c32, "c16": c16})
    return in_maps


def _unshard(outT):
    """[128, 16] -> [8, 256]: out[b, c*128+p] = outT[p, c*8+b]."""
    return np.ascontiguousarray(
        outT.reshape(P128, 2, BL).transpose(2, 1, 0).reshape(BL, OUT), dtype=np.float32
    )


def kernel(**inputs):
    from concourse.bass_utils import run_bass_kernel_spmd

    nc = get_nc()
    in_maps = host_prep(**inputs)
    res = run_bass_kernel_spmd(nc, in_maps, core_ids=list(range(NCORES)))
    out = np.concatenate(
        [_unshard(res.results[k]["outT"]) for k in range(NCORES)], axis=0
    )
    return np.ascontiguousarray(out, dtype=np.float32)
